# revision 53
# baseline (speedup 1.0000x reference)
"""Trainium2 Bass kernel for nn_AttentionFusion (B=8192, M=4, H=1024), 8-core data parallel.

Math (exact reformulation of the reference):
  scores[b,m,n] = conf[b,m] * (q_{4b+m} . k_{4b+n}) / sqrt(H)
                = conf[b,m] * (Y[4b+m] . X[4b+n] + alpha[4b+m] + beta[4b+n] + d)
      with Y = X G, G = (Wq/sqrt(H))^T Wk,
      alpha = X ((Wq/32)^T bk) + d, beta = X (Wk^T bq/32), d = (bq/32).bk
  wt[b,n] = sum_m softmax_n(scores)[b,m,n]      (convex weights * 4)
  Z[b]    = sum_n wt[b,n] X[4b+n]               (mean over m folds into Wc)
  out[b]  = Z[b] (Wo Wv / 4)^T + (bv Wo^T + bo)

TensorEngine: Y = X G in fp8e4 DoubleRow (G pre-scaled x64, undone in the
PSUM drain), per-128-token gram S = Y X^T in bf16 + K=2 rank-2 bias fixup,
out = Z Wc^T bf16.  Queue discipline (head-of-line blocking is the enemy):
sync ring carries only always-ready streams (inputs, prefetch, gram-outs,
stores); the scalar ring does startup loads; gpsimd does the dependent small
DMAs (diag gathers j2/j3, w_dr/wrep round trip); the gathers j0/j1 ride sync.
Scalar engine = PSUM drains + EXP + bias-adds (so out-proj banks recycle
promptly); DVE = softmax + z-combine only, with zc emitted BEFORE the
deferred out-proj so it never queues behind bias-adds.  Each super-tile is
half-pipelined (Y/gram/softmax/zc per 512 tokens), out-proj deferred one
super-tile, and the last super-tile runs per-half zc->out_proj.
"""
import sys

if '/opt/trn_rl_repo' not in sys.path:
    sys.path.insert(0, '/opt/trn_rl_repo')

import numpy as np
import ml_dtypes

B, M, H = 8192, 4, 1024
NCORES = 8
B_CORE = B // NCORES            # 1024 batch rows per core
T_CORE = B_CORE * M             # 4096 tokens per core
T_SUPER = 1024
N_SUPER = T_CORE // T_SUPER
SIZES = (1024, 1024, 1024, 1024)       # super-tile token counts (sum = T_CORE)
P = 128
OC = H // P                     # 8 output chunks
HC = H // P                     # 8 contraction chunks
NB = B_CORE // P                # 8 batch tiles of 128 rows
ZC_V = 5                        # z-combine chunks on vector (rest on gpsimd)
BF16 = ml_dtypes.bfloat16
FP8 = ml_dtypes.float8_e4m3
GSCALE = 64.0                   # G is scaled x64 into fp8e4 range; Y drain undoes it

_NC_CACHE = {}


def build_bass(sizes=SIZES):
    import concourse.bass as bass
    import concourse.mybir as mybir
    import concourse.tile as tile
    from concourse import bacc

    n_super = len(sizes)
    t_core = sum(sizes)
    assert t_core == T_CORE
    mts = max(sizes)             # max super-tile tokens
    mbs = mts // M               # max batch rows per super-tile
    DR = mybir.MatmulPerfMode.DoubleRow

    nc = bacc.Bacc(None, target_bir_lowering=False)
    x8 = nc.dram_tensor("x8", [P, HC, t_core], mybir.dt.float8e4, kind="ExternalInput")
    xb = nc.dram_tensor("xb", [P, HC, t_core], mybir.dt.bfloat16, kind="ExternalInput")
    wg8 = nc.dram_tensor("wg8", [P, HC, H], mybir.dt.float8e4, kind="ExternalInput")
    wc = nc.dram_tensor("wc", [P, HC, H], mybir.dt.bfloat16, kind="ExternalInput")
    aby = nc.dram_tensor("aby", [2, t_core], mybir.dt.bfloat16, kind="ExternalInput")
    abx = nc.dram_tensor("abx", [2, t_core], mybir.dt.bfloat16, kind="ExternalInput")
    bc = nc.dram_tensor("bc", [P, OC], mybir.dt.float32, kind="ExternalInput")
    conf = nc.dram_tensor("conf", [P, t_core // P], mybir.dt.float32,
                          kind="ExternalInput")
    bst = nc.dram_tensor("bst", [32, t_core // P, P], mybir.dt.bfloat16,
                         kind="ExternalInput")
    bmov = nc.dram_tensor("bmov", [32, P], mybir.dt.bfloat16, kind="ExternalInput")
    outT = nc.dram_tensor("outT", [H, B_CORE], mybir.dt.float32, kind="ExternalOutput")

    FDT = mybir.dt.float32
    F32R = mybir.dt.float32r
    BDT = mybir.dt.bfloat16
    QDT = mybir.dt.float8e4
    AX = mybir.AxisListType.X
    MUL = mybir.AluOpType.mult
    ADD = mybir.AluOpType.add

    from contextlib import ExitStack
    with tile.TileContext(nc) as tc:
        with ExitStack() as _es:
            wp = _es.enter_context(tc.tile_pool(name="wp", bufs=1))
            cp = _es.enter_context(tc.tile_pool(name="cp", bufs=1))
            x8p = _es.enter_context(tc.tile_pool(name="x8p", bufs=3))
            xbp = _es.enter_context(tc.tile_pool(name="xbp", bufs=3))
            abp = _es.enter_context(tc.tile_pool(name="abp", bufs=n_super))
            yp = _es.enter_context(tc.tile_pool(name="yp", bufs=2))
            zp = _es.enter_context(tc.tile_pool(name="zp", bufs=3))
            gpl = _es.enter_context(tc.tile_pool(name="gp", bufs=2))
            smp = _es.enter_context(tc.tile_pool(name="smp", bufs=2))
            wrp = _es.enter_context(tc.tile_pool(name="wrp", bufs=2))
            prp = _es.enter_context(tc.tile_pool(name="prp", bufs=4))
            osb = _es.enter_context(tc.tile_pool(name="osb", bufs=3))
            psp = _es.enter_context(tc.tile_pool(name="psp", bufs=3, space="PSUM"))
            psg = _es.enter_context(tc.tile_pool(name="psg", bufs=2, space="PSUM"))
            pso = _es.enter_context(tc.tile_pool(name="pso", bufs=2, space="PSUM"))
            psw = _es.enter_context(tc.tile_pool(name="psw", bufs=1, space="PSUM"))
            drg = _es.enter_context(tc.tile_pool(name="drg", bufs=4, space="DRAM"))
            drw = _es.enter_context(tc.tile_pool(name="drw", bufs=4, space="DRAM"))

            # ---- startup: wg8 chunks on sync ring, x8(super 0) chunks on the
            #      scalar ring so descriptor issue runs in parallel; warmup
            #      matmuls AFTER the DMA issues so no queue waits on the PE ----
            wg_sb = wp.tile([P, HC, H], QDT, tag="wg", name="wg_sb")
            x8ts = [None] * n_super
            xbts = [None] * n_super
            x8ts[0] = x8p.tile([P, HC, mts], QDT, tag="x8t", name="x8t0")
            for c in range(HC // 2):
                nc.sync.dma_start(wg_sb[:, 2 * c:2 * c + 2, :],
                                  wg8[:, 2 * c:2 * c + 2, :])
                nc.scalar.dma_start(x8ts[0][:, 2 * c:2 * c + 2, 0:sizes[0]],
                                    x8[:, 2 * c:2 * c + 2, 0:sizes[0]])
            xbts[0] = xbp.tile([P, HC, mts], BDT, tag="xbt", name="xbt0")
            nc.sync.dma_start(xbts[0][:, :, 0:512], xb[:, :, 0:512])
            nc.sync.dma_start(xbts[0][:, :, 512:sizes[0]], xb[:, :, 512:sizes[0]])
            wc_sb = wp.tile([P, HC, H], BDT, tag="wc", name="wc_sb")
            bc_sb = cp.tile([P, OC], FDT, tag="bc", name="bc_sb")
            nc.gpsimd.dma_start(bc_sb[:], bc[:])
            conf_sb = cp.tile([P, t_core // P], FDT, tag="conf", name="conf_sb")
            nc.gpsimd.dma_start(conf_sb[:], conf[:])
            bst_sb = cp.tile([32, t_core // P, P], BDT, tag="bst", name="bst_sb")
            nc.gpsimd.dma_start(bst_sb[:], bst[:])
            bmov_sb = cp.tile([32, P], BDT, tag="bmov", name="bmov_sb")
            nc.gpsimd.dma_start(bmov_sb[:], bmov[:])

            # HAM warmup: dataless matmuls keep the PE busy during the DMA
            # lead-in so the real Y stream starts near the warm clock
            wu = wp.tile([P, P], BDT, tag="warm", name="warm_sb")
            nc.vector.memset(wu[:], 1.0)
            wups = psg.tile([P, P], FDT, tag="gram_ps", name="warm_ps")
            for i in range(26):
                nc.tensor.matmul(wups[:], wu[:], wu[:],
                                 start=(i == 0), stop=(i == 25))
            wuo = wp.tile([P, P], FDT, tag="warmo", name="warm_out")
            nc.vector.tensor_copy(wuo[:], wups[:])

            def out_proj(zT, b0, bs, zlo=0):
                """out[:, b0:b0+bs] = Wc^T zT[:, :, zlo:zlo+bs] + bc."""
                o_all = osb.tile([P, OC, 256], FDT, tag="osb")
                for oc in range(OC):
                    pt = pso.tile([P, 256], FDT, tag="outp")
                    for hc in range(HC):
                        nc.tensor.matmul(
                            pt[:, :bs], wc_sb[:, hc, oc * P:(oc + 1) * P],
                            zT[:, hc, zlo:zlo + bs],
                            start=(hc == 0), stop=(hc == HC - 1))
                    nc.scalar.add(o_all[:, oc, :bs], pt[:, :bs],
                                  bc_sb[:, oc:oc + 1])
                nc.sync.dma_start(
                    outT[:, b0:b0 + bs].rearrange("(c p) b -> p c b", p=P),
                    o_all[:, :, :bs])

            def gram_exp_half(yT, xbt, ay, ax, st, gt0):
                """Gram + rank-2 bias/−K fixup + K=32 diag add-back, exp with
                conf scaling in the ACT drain (accum_out = row sums), then the
                m-summed softmax weights via a reciprocal-stationary column-sum
                matmul.  No DRAM gather round trip."""
                e_sb = gpl.tile([P, 4, P], BDT, tag="gram")
                z4a = smp.tile([P, 4], FDT, tag="z4a", name="z4a")
                for tt in range(4):
                    gps = psg.tile([P, P], FDT, tag="gram_ps")
                    tsl = slice(st * 512 + tt * P, st * 512 + (tt + 1) * P)
                    for oc in range(OC):
                        nc.tensor.matmul(gps[:], yT[:, oc, tsl], xbt[:, oc, tsl],
                                         start=(oc == 0), stop=False)
                    nc.tensor.matmul(gps[:], ay[:, tsl], ax[:, tsl],
                                     start=False, stop=False)
                    nc.tensor.matmul(gps[:], bst_sb[:, gt0 + tt, :], bmov_sb[:],
                                     start=False, stop=True)
                    nc.scalar.activation(
                        e_sb[:, tt, :], gps[:], mybir.ActivationFunctionType.Exp,
                        scale=conf_sb[:, gt0 + tt:gt0 + tt + 1],
                        accum_out=z4a[:, tt:tt + 1])
                r4 = smp.tile([P, 4], FDT, tag="r4", name="r4")
                nc.vector.reciprocal(r4[:], z4a[:])
                r4b = smp.tile([P, 4], BDT, tag="r4b", name="r4b")
                nc.vector.tensor_copy(r4b[:], r4[:])
                wps = psw.tile([1, 512], FDT, tag="wps")
                for tt in range(4):
                    nc.tensor.matmul(wps[0:1, tt * P:(tt + 1) * P],
                                     r4b[:, tt:tt + 1], e_sb[:, tt, :],
                                     start=True, stop=True)
                wfl = smp.tile([1, 512], BDT, tag="wfl", name="wfl")
                nc.scalar.copy(wfl[:], wps[:])
                w_dr = drw.tile([1, 512], BDT, tag="w_dr", name="w_dr")
                nc.gpsimd.dma_start(w_dr[:], wfl[:])
                wrep = wrp.tile([P, 2, 512], BDT, tag="wrep", name="wrep")
                wsrc = bass.AP(w_dr[:].tensor, w_dr[:].offset,
                               [[0, P], [0, 2], [1, 512]])
                nc.gpsimd.dma_start(wrep[:], wsrc)
                return wrep

            def z_combine(zT, xbt, wrep, st):
                """Z[:, b] = sum_n wt[b,n] X[:, 4b+n]; 2 hc chunks fused per op."""
                with nc.allow_low_precision(reason="4-term convex combine, fp32 acc"):
                    for h2 in range(HC // 2):
                        prod = prp.tile([P, 2, 512], BDT, tag="prod", name="prod")
                        nc.vector.tensor_mul(
                            prod[:],
                            xbt[:, 2 * h2:2 * h2 + 2, st * 512:(st + 1) * 512],
                            wrep[:])
                        nc.vector.reduce_sum(
                            zT[:, 2 * h2:2 * h2 + 2, st * P:(st + 1) * P],
                            prod[:].rearrange("p c (b n) -> p c b n", n=4), axis=AX)

            z_queue = []             # (zT, b0, bs) pending output projections
            T0 = 0
            for s, ts in enumerate(sizes):
                nst = ts // 512
                b0 = T0 // M
                bs = ts // M
                x8t, xbt = x8ts[s], xbts[s]
                ay = abp.tile([2, mts], BDT, tag="ay")
                nc.gpsimd.dma_start(ay[:, 0:ts], aby[:, T0:T0 + ts])
                ax = abp.tile([2, mts], BDT, tag="ax")
                nc.gpsimd.dma_start(ax[:, 0:ts], abx[:, T0:T0 + ts])

                yT = yp.tile([P, OC, mts], BDT, tag="yT")
                zT = zp.tile([P, HC, mbs], BDT, tag="zT")

                def y_half(st):
                    for oc in range(OC):
                        pt = psp.tile([P, 512], FDT, tag="proj")
                        for c in range(HC // 2):
                            nc.tensor.matmul(
                                pt[:], wg_sb[:, 2 * c:2 * c + 2, oc * P:(oc + 1) * P],
                                x8t[:, 2 * c:2 * c + 2, st * 512:(st + 1) * 512],
                                start=(c == 0), stop=(c == HC // 2 - 1),
                                perf_mode=DR)
                        nc.scalar.mul(yT[:, oc, st * 512:(st + 1) * 512], pt[:],
                                      1.0 / GSCALE)

                for st in range(nst):
                    y_half(st)
                    if st == nst - 1:
                        # prefetch next super-tile's activations on the sync ring
                        if s + 1 < n_super:
                            T1 = T0 + ts
                            ts1 = sizes[s + 1]
                            x8ts[s + 1] = x8p.tile([P, HC, mts], QDT, tag="x8t",
                                                   name=f"x8t{s+1}")
                            nc.sync.dma_start(x8ts[s + 1][:, :, 0:ts1],
                                              x8[:, :, T1:T1 + ts1])
                            xbts[s + 1] = xbp.tile([P, HC, mts], BDT, tag="xbt",
                                                   name=f"xbt{s+1}")
                            nc.sync.dma_start(xbts[s + 1][:, :, 0:ts1],
                                              xb[:, :, T1:T1 + ts1])
                        if s == 0:
                            nc.scalar.dma_start(wc_sb[:], wc[:])
                    wrep = gram_exp_half(yT, xbt, ay, ax, st,
                                         (T0 + st * 512) // P)
                    # zc emitted BEFORE the deferred out-proj: its DVE ops must
                    # not queue behind bias-adds waiting on out-proj matmuls
                    z_combine(zT, xbt, wrep, st)

                if s < n_super - 1:
                    if z_queue:
                        out_proj(*z_queue.pop(0))
                    z_queue.append((zT, b0, bs))
                else:
                    for zp_args in z_queue:
                        out_proj(*zp_args)
                    for st in range(nst):
                        out_proj(zT, b0 + st * 128, 128, zlo=st * 128)
                T0 += ts
    nc.compile()
    return nc


def _get_nc(sizes=SIZES):
    if sizes not in _NC_CACHE:
        _NC_CACHE[sizes] = build_bass(sizes)
    return _NC_CACHE[sizes]


def prep_in_maps(inputs, ncores=NCORES):
    """Host-side: fold weights, shard + transpose activations, cast bf16/fp8."""
    f32 = np.float32
    f64 = np.float64
    feats = np.asarray(inputs["features"], f32)
    confs = np.asarray(inputs["confidences"], f32).reshape(-1, M)
    Wq = np.asarray(inputs["Wq"], f64)
    Wk = np.asarray(inputs["Wk"], f64)
    Wv = np.asarray(inputs["Wv"], f64)
    Wo = np.asarray(inputs["Wo"], f64)
    bq = np.asarray(inputs["bq"], f64)
    bk = np.asarray(inputs["bk"], f64)
    bv = np.asarray(inputs["bv"], f64)
    bo = np.asarray(inputs["bo"], f64)

    s = 1.0 / np.sqrt(H)
    G = (Wq * s).T @ Wk                         # [h, h']
    wg8_h = np.ascontiguousarray(
        np.clip(G * GSCALE, -240, 240).reshape(HC, P, H).transpose(1, 0, 2)
    ).astype(FP8)
    wcT = ((Wo @ Wv) / 4.0).T                   # [h, o]
    wc_h = np.ascontiguousarray(
        wcT.reshape(HC, P, H).transpose(1, 0, 2)).astype(BF16)
    bc_full = (bv @ Wo.T + bo).astype(f32)
    bc_h = np.ascontiguousarray(bc_full.reshape(OC, P).T)
    u = (Wq * s).T @ bk                         # alpha = X u + d
    w_vec = Wk.T @ (bq * s)                     # beta = X w_vec
    d = float((bq * s) @ bk)

    nb = feats.shape[0]
    b_core = nb // ncores
    X = feats.reshape(nb * M, H)
    alpha = (X @ u.astype(f32) + f32(d)).astype(f32)
    beta = (X @ w_vec.astype(f32)).astype(f32)
    t_core = b_core * M

    in_maps = []
    for c in range(ncores):
        tsl = slice(c * t_core, (c + 1) * t_core)
        XT = np.ascontiguousarray(X[tsl].T)     # [H, t_core]
        xb_h = np.ascontiguousarray(
            XT.reshape(HC, P, t_core).transpose(1, 0, 2)).astype(BF16)
        x8_h = np.ascontiguousarray(
            np.clip(XT, -240, 240).reshape(HC, P, t_core).transpose(1, 0, 2)
        ).astype(FP8)
        ab_y = np.ones((2, t_core), f32)
        ab_y[0] = alpha[tsl]
        ab_x = np.ones((2, t_core), f32)
        ab_x[1] = beta[tsl]
        conf_t = confs[c * b_core:(c + 1) * b_core].reshape(-1)  # token-major
        conf_t = np.repeat(confs[c * b_core:(c + 1) * b_core], 1, axis=0).reshape(-1)
        conf_tok = np.ascontiguousarray(
            conf_t.reshape(t_core // P, P).T)   # [P, ntiles]
        kb = (50.0 / np.maximum(conf_t, 1e-6)).astype(BF16)   # per-token mask offset
        ab_y[0] = -kb.astype(f32)               # replaces alpha (softmax-invariant)
        nt = t_core // P
        gidx = (np.arange(P) // 4)              # token-in-tile -> group
        bst_h = np.zeros((32, nt, P), f32)
        bst_h[gidx, :, np.arange(P)] = kb.astype(f32).reshape(nt, P).T
        bmov_h = (gidx[None, :] == np.arange(32)[:, None]).astype(f32)
        in_maps.append({
            "x8": x8_h, "xb": xb_h, "wg8": wg8_h, "wc": wc_h, "bc": bc_h,
            "aby": ab_y.astype(BF16), "abx": ab_x.astype(BF16),
            "conf": conf_tok,
            "bst": np.ascontiguousarray(bst_h).astype(BF16),
            "bmov": np.ascontiguousarray(bmov_h).astype(BF16),
        })
    return in_maps


def install_ntff_hook():
    """Best-effort shim so run_bass_kernel_spmd(trace=True) can profile under axon."""
    import types
    try:
        from antenv.axon_hooks import get_axon_ntff_profile_hook  # noqa: F401
        return True
    except ImportError:
        pass
    try:
        import antenv
        mod = types.ModuleType("antenv.axon_hooks")
        _state = {"hook": None}
        mod.set_axon_ntff_profile_hook = lambda h: _state.__setitem__("hook", h)
        mod.get_axon_ntff_profile_hook = lambda: _state["hook"]
        sys.modules["antenv.axon_hooks"] = mod
        antenv.axon_hooks = mod
        from trn_agent_boot.trn_boot import _ntff_profile_via_ctypes
        hook = _ntff_profile_via_ctypes('/opt/axon/libaxon_pjrt.so')
        if hook is None:
            return False
        mod.set_axon_ntff_profile_hook(hook)
        return True
    except Exception:
        return False


def run(inputs, trace=False, tmpdir=None):
    """Run the 8-core kernel; returns (out [B, H] f32, BassKernelResults)."""
    from concourse.bass_utils import run_bass_kernel_spmd
    nc = _get_nc()
    in_maps = prep_in_maps(inputs)
    if trace:
        install_ntff_hook()
    res = run_bass_kernel_spmd(nc, in_maps, core_ids=list(range(NCORES)),
                               trace=trace, tmpdir=tmpdir)
    out = np.concatenate(
        [np.asarray(o["outT"], np.float32).T for o in res.results], axis=0)
    return out, res


def kernel(**inputs):
    out, _ = run(inputs, trace=False)
    return out


# revision 55
# speedup vs baseline: 1.0815x; 1.0815x over previous
"""Trainium2 Bass kernel for nn_AttentionFusion (B=8192, M=4, H=1024), 8-core data parallel.

Math (exact reformulation of the reference):
  scores[b,m,n] = conf[b,m] * (q_{4b+m} . k_{4b+n}) / sqrt(H)
                = conf[b,m] * (Y[4b+m] . X[4b+n] + alpha[4b+m] + beta[4b+n] + d)
      with Y = X G, G = (Wq/sqrt(H))^T Wk,
      alpha = X ((Wq/32)^T bk) + d, beta = X (Wk^T bq/32), d = (bq/32).bk
  wt[b,n] = sum_m softmax_n(scores)[b,m,n]      (convex weights * 4)
  Z[b]    = sum_n wt[b,n] X[4b+n]               (mean over m folds into Wc)
  out[b]  = Z[b] (Wo Wv / 4)^T + (bv Wo^T + bo)

TensorEngine: Y = X G in fp8e4 DoubleRow (G pre-scaled x64, undone in the
PSUM drain), per-128-token gram S = Y X^T in bf16 + K=2 rank-2 bias fixup,
out = Z Wc^T bf16.  Queue discipline (head-of-line blocking is the enemy):
sync ring carries only always-ready streams (inputs, prefetch, gram-outs,
stores); the scalar ring does startup loads; gpsimd does the dependent small
DMAs (diag gathers j2/j3, w_dr/wrep round trip); the gathers j0/j1 ride sync.
Scalar engine = PSUM drains + EXP + bias-adds (so out-proj banks recycle
promptly); DVE = softmax + z-combine only, with zc emitted BEFORE the
deferred out-proj so it never queues behind bias-adds.  Each super-tile is
half-pipelined (Y/gram/softmax/zc per 512 tokens), out-proj deferred one
super-tile, and the last super-tile runs per-half zc->out_proj.
"""
import sys

if '/opt/trn_rl_repo' not in sys.path:
    sys.path.insert(0, '/opt/trn_rl_repo')

import numpy as np
import ml_dtypes

B, M, H = 8192, 4, 1024
NCORES = 8
B_CORE = B // NCORES            # 1024 batch rows per core
T_CORE = B_CORE * M             # 4096 tokens per core
T_SUPER = 1024
N_SUPER = T_CORE // T_SUPER
SIZES = (1024, 1024, 1024, 1024)       # super-tile token counts (sum = T_CORE)
P = 128
OC = H // P                     # 8 output chunks
HC = H // P                     # 8 contraction chunks
NB = B_CORE // P                # 8 batch tiles of 128 rows
ZC_V = 5                        # z-combine chunks on vector (rest on gpsimd)
BF16 = ml_dtypes.bfloat16
FP8 = ml_dtypes.float8_e4m3
GSCALE = 64.0                   # G is scaled x64 into fp8e4 range; Y drain undoes it

_NC_CACHE = {}


def build_bass(sizes=SIZES):
    import concourse.bass as bass
    import concourse.mybir as mybir
    import concourse.tile as tile
    from concourse import bacc

    n_super = len(sizes)
    t_core = sum(sizes)
    assert t_core == T_CORE
    mts = max(sizes)             # max super-tile tokens
    mbs = mts // M               # max batch rows per super-tile
    DR = mybir.MatmulPerfMode.DoubleRow

    nc = bacc.Bacc(None, target_bir_lowering=False)
    x8 = nc.dram_tensor("x8", [P, HC, t_core], mybir.dt.float8e4, kind="ExternalInput")
    xb = nc.dram_tensor("xb", [P, HC, t_core], mybir.dt.bfloat16, kind="ExternalInput")
    wg8 = nc.dram_tensor("wg8", [P, HC, H], mybir.dt.float8e4, kind="ExternalInput")
    wc = nc.dram_tensor("wc", [P, HC, H], mybir.dt.bfloat16, kind="ExternalInput")
    aby = nc.dram_tensor("aby", [2, t_core], mybir.dt.bfloat16, kind="ExternalInput")
    abx = nc.dram_tensor("abx", [2, t_core], mybir.dt.bfloat16, kind="ExternalInput")
    bc = nc.dram_tensor("bc", [P, OC], mybir.dt.float32, kind="ExternalInput")
    conf = nc.dram_tensor("conf", [P, t_core // P], mybir.dt.float32,
                          kind="ExternalInput")
    bst = nc.dram_tensor("bst", [32, t_core // P, P], mybir.dt.bfloat16,
                         kind="ExternalInput")
    bmov = nc.dram_tensor("bmov", [32, P], mybir.dt.bfloat16, kind="ExternalInput")
    outT = nc.dram_tensor("outT", [H, B_CORE], mybir.dt.float32, kind="ExternalOutput")

    FDT = mybir.dt.float32
    F32R = mybir.dt.float32r
    BDT = mybir.dt.bfloat16
    QDT = mybir.dt.float8e4
    AX = mybir.AxisListType.X
    MUL = mybir.AluOpType.mult
    ADD = mybir.AluOpType.add

    from contextlib import ExitStack
    with tile.TileContext(nc) as tc:
        with ExitStack() as _es:
            wp = _es.enter_context(tc.tile_pool(name="wp", bufs=1))
            cp = _es.enter_context(tc.tile_pool(name="cp", bufs=1))
            x8p = _es.enter_context(tc.tile_pool(name="x8p", bufs=3))
            xbp = _es.enter_context(tc.tile_pool(name="xbp", bufs=3))
            abp = _es.enter_context(tc.tile_pool(name="abp", bufs=n_super))
            yp = _es.enter_context(tc.tile_pool(name="yp", bufs=2))
            zp = _es.enter_context(tc.tile_pool(name="zp", bufs=3))
            gpl = _es.enter_context(tc.tile_pool(name="gp", bufs=2))
            smp = _es.enter_context(tc.tile_pool(name="smp", bufs=2))
            wrp = _es.enter_context(tc.tile_pool(name="wrp", bufs=2))
            prp = _es.enter_context(tc.tile_pool(name="prp", bufs=4))
            osb = _es.enter_context(tc.tile_pool(name="osb", bufs=3))
            psp = _es.enter_context(tc.tile_pool(name="psp", bufs=4, space="PSUM"))
            psg = _es.enter_context(tc.tile_pool(name="psg", bufs=2, space="PSUM"))
            pso = _es.enter_context(tc.tile_pool(name="pso", bufs=2, space="PSUM"))
            drg = _es.enter_context(tc.tile_pool(name="drg", bufs=4, space="DRAM"))
            drw = _es.enter_context(tc.tile_pool(name="drw", bufs=4, space="DRAM"))

            # ---- startup: wg8 chunks on sync ring, x8(super 0) chunks on the
            #      scalar ring so descriptor issue runs in parallel; warmup
            #      matmuls AFTER the DMA issues so no queue waits on the PE ----
            wg_sb = wp.tile([P, HC, H], QDT, tag="wg", name="wg_sb")
            x8ts = [None] * n_super
            xbts = [None] * n_super
            x8ts[0] = x8p.tile([P, HC, mts], QDT, tag="x8t", name="x8t0")
            for c in range(HC // 2):
                nc.sync.dma_start(wg_sb[:, 2 * c:2 * c + 2, :],
                                  wg8[:, 2 * c:2 * c + 2, :])
                nc.scalar.dma_start(x8ts[0][:, 2 * c:2 * c + 2, 0:sizes[0]],
                                    x8[:, 2 * c:2 * c + 2, 0:sizes[0]])
            xbts[0] = xbp.tile([P, HC, mts], BDT, tag="xbt", name="xbt0")
            nc.sync.dma_start(xbts[0][:, :, 0:512], xb[:, :, 0:512])
            nc.sync.dma_start(xbts[0][:, :, 512:sizes[0]], xb[:, :, 512:sizes[0]])
            wc_sb = wp.tile([P, HC, H], BDT, tag="wc", name="wc_sb")
            bc_sb = cp.tile([P, OC], FDT, tag="bc", name="bc_sb")
            nc.gpsimd.dma_start(bc_sb[:], bc[:])
            conf_sb = cp.tile([P, t_core // P], FDT, tag="conf", name="conf_sb")
            nc.gpsimd.dma_start(conf_sb[:], conf[:])
            bst_sb = cp.tile([32, t_core // P, P], BDT, tag="bst", name="bst_sb")
            nc.gpsimd.dma_start(bst_sb[:], bst[:])
            bmov_sb = cp.tile([32, P], BDT, tag="bmov", name="bmov_sb")
            nc.gpsimd.dma_start(bmov_sb[:], bmov[:])

            # HAM warmup: dataless matmuls keep the PE busy during the DMA
            # lead-in so the real Y stream starts near the warm clock
            wu = wp.tile([P, P], BDT, tag="warm", name="warm_sb")
            nc.vector.memset(wu[:], 1.0)
            wups = psg.tile([P, P], FDT, tag="gram_ps", name="warm_ps")
            for i in range(26):
                nc.tensor.matmul(wups[:], wu[:], wu[:],
                                 start=(i == 0), stop=(i == 25))
            wuo = wp.tile([P, P], FDT, tag="warmo", name="warm_out")
            nc.vector.tensor_copy(wuo[:], wups[:])

            def out_proj(zT, b0, bs, zlo=0):
                """out[:, b0:b0+bs] = Wc^T zT[:, :, zlo:zlo+bs] + bc."""
                o_all = osb.tile([P, OC, 256], FDT, tag="osb")
                for oc in range(OC):
                    pt = pso.tile([P, 256], FDT, tag="outp")
                    for hc in range(HC):
                        nc.tensor.matmul(
                            pt[:, :bs], wc_sb[:, hc, oc * P:(oc + 1) * P],
                            zT[:, hc, zlo:zlo + bs],
                            start=(hc == 0), stop=(hc == HC - 1))
                    nc.scalar.add(o_all[:, oc, :bs], pt[:, :bs],
                                  bc_sb[:, oc:oc + 1])
                nc.sync.dma_start(
                    outT[:, b0:b0 + bs].rearrange("(c p) b -> p c b", p=P),
                    o_all[:, :, :bs])

            def gram_exp_half(yT, xbt, ay, ax, st, gt0):
                """Gram + rank-2 bias/−K fixup + K=32 diag add-back, exp with
                conf scaling in the ACT drain (accum_out = row sums), then the
                m-summed softmax weights via a reciprocal-stationary column-sum
                matmul.  No DRAM gather round trip."""
                e_sb = gpl.tile([P, 4, P], BDT, tag="gram")
                z4a = smp.tile([P, 4], FDT, tag="z4a", name="z4a")
                for tt in range(4):
                    gps = psg.tile([P, P], FDT, tag="gram_ps")
                    tsl = slice(st * 512 + tt * P, st * 512 + (tt + 1) * P)
                    for oc in range(OC):
                        nc.tensor.matmul(gps[:], yT[:, oc, tsl], xbt[:, oc, tsl],
                                         start=(oc == 0), stop=False)
                    nc.tensor.matmul(gps[:], ay[:, tsl], ax[:, tsl],
                                     start=False, stop=False)
                    nc.tensor.matmul(gps[:], bst_sb[:, gt0 + tt, :], bmov_sb[:],
                                     start=False, stop=True)
                    nc.scalar.activation(
                        e_sb[:, tt, :], gps[:], mybir.ActivationFunctionType.Exp,
                        scale=conf_sb[:, gt0 + tt:gt0 + tt + 1],
                        accum_out=z4a[:, tt:tt + 1])
                r4 = smp.tile([P, 4], FDT, tag="r4", name="r4")
                nc.vector.reciprocal(r4[:], z4a[:])
                r4b = smp.tile([P, 4], BDT, tag="r4b", name="r4b")
                nc.vector.tensor_copy(r4b[:], r4[:])
                return e_sb, r4b

            def colsum_w(e_sb, r4b):
                """Softmax weights = reciprocal-stationary column sums; emitted
                AFTER later PE work so the recip never stalls the PE FIFO."""
                wps = psp.tile([P, 512], FDT, tag="proj", name="wps")
                for tt in range(4):
                    nc.tensor.matmul(wps[0:1, tt * P:(tt + 1) * P],
                                     r4b[:, tt:tt + 1], e_sb[:, tt, :],
                                     start=True, stop=True)
                wfl = smp.tile([1, 512], BDT, tag="wfl", name="wfl")
                nc.scalar.copy(wfl[:], wps[0:1, :])
                w_dr = drw.tile([1, 512], BDT, tag="w_dr", name="w_dr")
                nc.gpsimd.dma_start(w_dr[:], wfl[:])
                wrep = wrp.tile([P, 2, 512], BDT, tag="wrep", name="wrep")
                wsrc = bass.AP(w_dr[:].tensor, w_dr[:].offset,
                               [[0, P], [0, 2], [1, 512]])
                nc.gpsimd.dma_start(wrep[:], wsrc)
                return wrep

            def z_combine(zT, xbt, wrep, st):
                """Z[:, b] = sum_n wt[b,n] X[:, 4b+n]; 2 hc chunks fused per op."""
                with nc.allow_low_precision(reason="4-term convex combine, fp32 acc"):
                    for h2 in range(HC // 2):
                        prod = prp.tile([P, 2, 512], BDT, tag="prod", name="prod")
                        nc.vector.tensor_mul(
                            prod[:],
                            xbt[:, 2 * h2:2 * h2 + 2, st * 512:(st + 1) * 512],
                            wrep[:])
                        nc.vector.reduce_sum(
                            zT[:, 2 * h2:2 * h2 + 2, st * P:(st + 1) * P],
                            prod[:].rearrange("p c (b n) -> p c b n", n=4), axis=AX)

            z_queue = []             # (zT, b0, bs) pending output projections
            T0 = 0
            for s, ts in enumerate(sizes):
                nst = ts // 512
                b0 = T0 // M
                bs = ts // M
                x8t, xbt = x8ts[s], xbts[s]
                ay = abp.tile([2, mts], BDT, tag="ay")
                nc.gpsimd.dma_start(ay[:, 0:ts], aby[:, T0:T0 + ts])
                ax = abp.tile([2, mts], BDT, tag="ax")
                nc.gpsimd.dma_start(ax[:, 0:ts], abx[:, T0:T0 + ts])

                yT = yp.tile([P, OC, mts], BDT, tag="yT")
                zT = zp.tile([P, HC, mbs], BDT, tag="zT")

                def y_half(st):
                    for oc in range(OC):
                        pt = psp.tile([P, 512], FDT, tag="proj")
                        for c in range(HC // 2):
                            nc.tensor.matmul(
                                pt[:], wg_sb[:, 2 * c:2 * c + 2, oc * P:(oc + 1) * P],
                                x8t[:, 2 * c:2 * c + 2, st * 512:(st + 1) * 512],
                                start=(c == 0), stop=(c == HC // 2 - 1),
                                perf_mode=DR)
                        nc.scalar.mul(yT[:, oc, st * 512:(st + 1) * 512], pt[:],
                                      1.0 / GSCALE)

                y_half(0)
                e0, r0 = gram_exp_half(yT, xbt, ay, ax, 0, T0 // P)
                y_half(1)
                # prefetch next super-tile's activations on the sync ring
                if s + 1 < n_super:
                    T1 = T0 + ts
                    ts1 = sizes[s + 1]
                    x8ts[s + 1] = x8p.tile([P, HC, mts], QDT, tag="x8t",
                                           name=f"x8t{s+1}")
                    nc.sync.dma_start(x8ts[s + 1][:, :, 0:ts1],
                                      x8[:, :, T1:T1 + ts1])
                    xbts[s + 1] = xbp.tile([P, HC, mts], BDT, tag="xbt",
                                           name=f"xbt{s+1}")
                    nc.sync.dma_start(xbts[s + 1][:, :, 0:ts1],
                                      xb[:, :, T1:T1 + ts1])
                if s == 0:
                    nc.scalar.dma_start(wc_sb[:], wc[:])
                wrep0 = colsum_w(e0, r0)
                z_combine(zT, xbt, wrep0, 0)
                e1, r1 = gram_exp_half(yT, xbt, ay, ax, 1, T0 // P + 4)
                if s < n_super - 1:
                    if z_queue:
                        out_proj(*z_queue.pop(0))
                    wrep1 = colsum_w(e1, r1)
                    z_combine(zT, xbt, wrep1, 1)
                    z_queue.append((zT, b0, bs))
                else:
                    for zp_args in z_queue:
                        out_proj(*zp_args)
                    wrep1 = colsum_w(e1, r1)
                    z_combine(zT, xbt, wrep1, 1)
                    out_proj(zT, b0, 128, zlo=0)
                    out_proj(zT, b0 + 128, 128, zlo=128)
                T0 += ts
    nc.compile()
    return nc


def _get_nc(sizes=SIZES):
    if sizes not in _NC_CACHE:
        _NC_CACHE[sizes] = build_bass(sizes)
    return _NC_CACHE[sizes]


def prep_in_maps(inputs, ncores=NCORES):
    """Host-side: fold weights, shard + transpose activations, cast bf16/fp8."""
    f32 = np.float32
    f64 = np.float64
    feats = np.asarray(inputs["features"], f32)
    confs = np.asarray(inputs["confidences"], f32).reshape(-1, M)
    Wq = np.asarray(inputs["Wq"], f64)
    Wk = np.asarray(inputs["Wk"], f64)
    Wv = np.asarray(inputs["Wv"], f64)
    Wo = np.asarray(inputs["Wo"], f64)
    bq = np.asarray(inputs["bq"], f64)
    bk = np.asarray(inputs["bk"], f64)
    bv = np.asarray(inputs["bv"], f64)
    bo = np.asarray(inputs["bo"], f64)

    s = 1.0 / np.sqrt(H)
    G = (Wq * s).T @ Wk                         # [h, h']
    wg8_h = np.ascontiguousarray(
        np.clip(G * GSCALE, -240, 240).reshape(HC, P, H).transpose(1, 0, 2)
    ).astype(FP8)
    wcT = ((Wo @ Wv) / 4.0).T                   # [h, o]
    wc_h = np.ascontiguousarray(
        wcT.reshape(HC, P, H).transpose(1, 0, 2)).astype(BF16)
    bc_full = (bv @ Wo.T + bo).astype(f32)
    bc_h = np.ascontiguousarray(bc_full.reshape(OC, P).T)
    u = (Wq * s).T @ bk                         # alpha = X u + d
    w_vec = Wk.T @ (bq * s)                     # beta = X w_vec
    d = float((bq * s) @ bk)

    nb = feats.shape[0]
    b_core = nb // ncores
    X = feats.reshape(nb * M, H)
    alpha = (X @ u.astype(f32) + f32(d)).astype(f32)
    beta = (X @ w_vec.astype(f32)).astype(f32)
    t_core = b_core * M

    in_maps = []
    for c in range(ncores):
        tsl = slice(c * t_core, (c + 1) * t_core)
        XT = np.ascontiguousarray(X[tsl].T)     # [H, t_core]
        xb_h = np.ascontiguousarray(
            XT.reshape(HC, P, t_core).transpose(1, 0, 2)).astype(BF16)
        x8_h = np.ascontiguousarray(
            np.clip(XT, -240, 240).reshape(HC, P, t_core).transpose(1, 0, 2)
        ).astype(FP8)
        ab_y = np.ones((2, t_core), f32)
        ab_y[0] = alpha[tsl]
        ab_x = np.ones((2, t_core), f32)
        ab_x[1] = beta[tsl]
        conf_t = confs[c * b_core:(c + 1) * b_core].reshape(-1)  # token-major
        conf_t = np.repeat(confs[c * b_core:(c + 1) * b_core], 1, axis=0).reshape(-1)
        conf_tok = np.ascontiguousarray(
            conf_t.reshape(t_core // P, P).T)   # [P, ntiles]
        kb = (50.0 / np.maximum(conf_t, 1e-6)).astype(BF16)   # per-token mask offset
        ab_y[0] = -kb.astype(f32)               # replaces alpha (softmax-invariant)
        nt = t_core // P
        gidx = (np.arange(P) // 4)              # token-in-tile -> group
        bst_h = np.zeros((32, nt, P), f32)
        bst_h[gidx, :, np.arange(P)] = kb.astype(f32).reshape(nt, P).T
        bmov_h = (gidx[None, :] == np.arange(32)[:, None]).astype(f32)
        in_maps.append({
            "x8": x8_h, "xb": xb_h, "wg8": wg8_h, "wc": wc_h, "bc": bc_h,
            "aby": ab_y.astype(BF16), "abx": ab_x.astype(BF16),
            "conf": conf_tok,
            "bst": np.ascontiguousarray(bst_h).astype(BF16),
            "bmov": np.ascontiguousarray(bmov_h).astype(BF16),
        })
    return in_maps


def install_ntff_hook():
    """Best-effort shim so run_bass_kernel_spmd(trace=True) can profile under axon."""
    import types
    try:
        from antenv.axon_hooks import get_axon_ntff_profile_hook  # noqa: F401
        return True
    except ImportError:
        pass
    try:
        import antenv
        mod = types.ModuleType("antenv.axon_hooks")
        _state = {"hook": None}
        mod.set_axon_ntff_profile_hook = lambda h: _state.__setitem__("hook", h)
        mod.get_axon_ntff_profile_hook = lambda: _state["hook"]
        sys.modules["antenv.axon_hooks"] = mod
        antenv.axon_hooks = mod
        from trn_agent_boot.trn_boot import _ntff_profile_via_ctypes
        hook = _ntff_profile_via_ctypes('/opt/axon/libaxon_pjrt.so')
        if hook is None:
            return False
        mod.set_axon_ntff_profile_hook(hook)
        return True
    except Exception:
        return False


def run(inputs, trace=False, tmpdir=None):
    """Run the 8-core kernel; returns (out [B, H] f32, BassKernelResults)."""
    from concourse.bass_utils import run_bass_kernel_spmd
    nc = _get_nc()
    in_maps = prep_in_maps(inputs)
    if trace:
        install_ntff_hook()
    res = run_bass_kernel_spmd(nc, in_maps, core_ids=list(range(NCORES)),
                               trace=trace, tmpdir=tmpdir)
    out = np.concatenate(
        [np.asarray(o["outT"], np.float32).T for o in res.results], axis=0)
    return out, res


def kernel(**inputs):
    out, _ = run(inputs, trace=False)
    return out
